# revision 8
# baseline (speedup 1.0000x reference)
"""Trainium2 Bass kernel for the BinaryLayer problem.

Math: out[b,o] = OR_r ( mask[o,r] AND AND_t x_in[b, w[o,r,t]] ) with
x_in = [1 | x | 1-x].  AND over 16 literals == (sum of literal values == 16).
sum_t lit = base[j] + sum_f C[f,j]*x[b,f]  where for j=(o,r):
  C[f,j]  = (#slots with w==f+1) - (#slots with w==f+1+F)
  base[j] = (#slots with w==0) + (#slots with w>F)
Fold threshold+mask into a constant row: c1[j] = base[j]-16 (valid term)
or base[j]-64 (padded term, all w==0).  Then with S[b,j] = x_aug[b,:]@A[:,j]
(A = [C; c1], x_aug = [x, 1]):  AND true <=> S==0, and since S<=0 always,
out[b,o] = (max_r S[b,o*32+r] >= 0).  All arithmetic is exact small-int
in fp8e4m3 inputs / f32 PSUM accumulation.

Sharding: 4 batch shards x 2 column shards across the 8 cores (core
c = (c//2)-th batch quarter, (c%2)-th j half); cuts per-core HBM bytes
to x 0.5MB + A 2MB vs 0.5+4 for pure data parallel.

Per core: x^T and A ship as fp8 already (host casts the 0/1 ints and
builds A), zero-padded to K=1024 rows so all 4 contraction passes are
uniform DoubleRow subtile pairs and the device needs no masking/casts.
PE runs 8 rounds of (4 passes x 4 batch tiles) fp8 DR matmuls into
4-bank PSUM tiles (107ns per 512-col matmul warm, ~13.7us total - the
critical path).  Draining 2M PSUM values is the second wall (~21us if
DVE did it all, the old bottleneck), so each round's 4 banks split
across all three other engines inside the PE round time:
  - banks 0-1 ("raw" pair): DVE 2-bank tensor_reduce(max) off PSUM;
    values stay ints <= 0 with 0 == term-true.
  - banks 2-3 ("indicator" pair): ACT computes relu(S+1) (exactly the
    0/1 AND-term indicator) PSUM->SBUF bf16; DVE folds 32->16 with one
    2x-mode bf16 tensor_max; gpsimd - which can neither reduce along
    the free axis nor touch PSUM, but CAN pairwise-add in SBUF - sums
    the 16 indicators with a 4-level add tree (OR == count nonzero).
Final compare on ACT: relu(y+1) for raw pairs, relu(y) for indicator
pairs (counts pass through; any nonzero uint8 is True for the caller).
The last round drains on DVE with compares + two half-DMAs chasing at
batch-tile grain to shorten the post-matmul tail.

DMA chunks are sized ~256KB (>=512B contiguous runs for full DMA rate)
and issued in exact consumption order so the PE never stalls after its
first real matmul; dummy matmuls on zeroed scratch cover the prologue
so the PE clock ramp (3us to full speed) completes as the data lands.
"""

import os

os.environ.setdefault("MYCRO_LOCAL_CACHE", "1")

import numpy as np

import concourse.bass as bass
import concourse.bacc as bacc
import concourse.mybir as mybir
from concourse.tile import TileContext
from concourse.bass_utils import run_bass_kernel_spmd

B, F = 4096, 784
OUT, OR_T, AND_T = 128, 32, 16
N_CORES = 8
JC = 2                       # j-dimension shards
BC = N_CORES // JC           # batch shards
BS = B // BC                 # 1024 batch rows per core
NBT = BS // 128              # 8 batch tiles of 128
J = OUT * OR_T               # 4096 (o,r) columns, j = o*32 + r
JS = J // JC                 # 2048 j columns per core
NJB = JS // 512              # 4 column blocks (one f32 PSUM bank each)
OJ = JS // OR_T              # 64 outputs per core
K = F + 1                    # 785 = 784 features + constant row
KPAD = 1024                  # padded to 8 full 128-row subtiles
KSUB = KPAD // 128           # 8
NP = KSUB // 2               # 4 DoubleRow passes
FP8 = mybir.dt.float8e4
FP8_NP = mybir.dt.np(FP8)
F32 = mybir.dt.float32
BF16 = mybir.dt.bfloat16
RELU = mybir.ActivationFunctionType.Relu
N_WU = 8                     # PE warm-up dummy matmuls

_CACHE: dict = {}


def _build_nc(use_double_row: bool = True) -> bass.Bass:
    nc = bacc.Bacc("TRN2")
    xT_d = nc.declare_dram_parameter("xT", [KPAD, BS], FP8, isOutput=False)
    A_d = nc.declare_dram_parameter("A", [KPAD, JS], FP8, isOutput=False)
    y_d = nc.declare_dram_parameter("y", [BS, OJ], mybir.dt.uint8, isOutput=True)

    with TileContext(nc) as tc:
        with (
            tc.tile_pool(name="const", bufs=1) as cpool,
            tc.tile_pool(name="psum", bufs=2, space="PSUM") as ppool,
            tc.tile_pool(name="tree", bufs=2) as tpool,
        ):
            A_sb = cpool.tile([128, KSUB, JS], FP8)
            x_q = cpool.tile([128, KSUB, BS], FP8)
            wu_sb = cpool.tile([128, 640], FP8)
            # Per-pair result accumulators [128, 2bt, jb, 16]: raw pairs
            # (bts 4g+0,1) f32 maxes; indicator pairs (bts 4g+2,3) bf16
            # counts/indicators (nonzero == True).
            y_fr = [cpool.tile([128, 2, NJB, 16], F32, name=f"y_fr{g}") for g in range(2)]
            y_fi = [cpool.tile([128, 2, NJB, 16], BF16, name=f"y_fi{g}") for g in range(2)]
            y_us = [cpool.tile([128, BC, OJ], mybir.dt.uint8, name=f"y_u{g}") for g in range(2)]
            scr_u = cpool.tile([128, 16], mybir.dt.uint8, name="scr_u")

            # Zero the PE warm-up scratch on gpsimd, and preload the ACT
            # Relu table early so the first real activation doesn't pay
            # the 1.3us table load.
            nc.gpsimd.memset(wu_sb[:], 0.0)
            nc.scalar.activation(
                out=scr_u[:], in_=wu_sb[:, 0:16], func=RELU, bias=1.0, scale=1.0
            )

            # DMA chunks in exact consumption order.  Rounds are
            # (jb, bt-group); passes consume x/A subtile pairs in order, so
            # ship [rows 0:512] then [512:1024] halves (~256KB each); all
            # contiguous runs are 512B so the DMA bus runs at full rate.
            def xdma(h, g):
                rows = slice(512 * h, 512 * (h + 1))
                cols = slice(512 * g, 512 * (g + 1))
                nc.sync.dma_start(
                    out=x_q[:, 4 * h : 4 * h + 4, cols],
                    in_=xT_d[rows, cols].rearrange("(s p) b -> p s b", p=128),
                )

            def adma(jb, h):
                rows = slice(512 * h, 512 * (h + 1))
                cols = slice(512 * jb, 512 * (jb + 1))
                nc.sync.dma_start(
                    out=A_sb[:, 4 * h : 4 * h + 4, cols],
                    in_=A_d[rows, cols].rearrange("(s p) j -> p s j", p=128),
                )

            xdma(0, 0)
            adma(0, 0)
            xdma(1, 0)
            adma(0, 1)
            xdma(0, 1)
            xdma(1, 1)
            for jb in range(1, NJB):
                adma(jb, 0)
                adma(jb, 1)

            # Dummy matmuls on zeroed scratch: keep the PE busy (the clock
            # ramp needs ~3us of sustained work to reach 2.4GHz) while the
            # first x/A chunks are in flight.  wtile is created before the
            # first round tile so the 2-buffer PSUM rotation pairs rounds
            # (r, r+2), not (r, r+1).
            wtile = ppool.tile([128, 4, 16, 32], F32, name="wu_ps", tag="ps")
            for _ in range(N_WU):
                nc.tensor.matmul(
                    wtile[:, 0], wu_sb[:, 0:128], wu_sb[:, 128:640], start=True, stop=True
                )

            rounds = [(jb, g) for jb in range(NJB) for g in range(2)]
            for jb, g in rounds:
                last = (jb, g) == rounds[-1]
                ptile = ppool.tile([128, 4, 16, 32], F32, name="ps", tag="ps")
                jsl = slice(512 * jb, 512 * (jb + 1))
                # Pass-major order matches the DMA chunk arrival order; the
                # last round is bt-major so its banks finish staggered and
                # the drain/compare/store chain starts early.
                mms = [(p, i) for p in range(NP) for i in range(4)]
                if last:
                    mms = [(p, i) for i in range(4) for p in range(NP)]
                for p, i in mms:
                    ssl = slice(2 * p, 2 * p + 2)
                    bt = 4 * g + i
                    bsl = slice(128 * bt, 128 * (bt + 1))
                    nc.tensor.matmul(
                        ptile[:, i],
                        x_q[:, ssl, bsl],
                        A_sb[:, ssl, jsl],
                        start=(p == 0),
                        stop=(p == NP - 1),
                        perf_mode=mybir.MatmulPerfMode.DoubleRow,
                    )
                # Raw pair (banks 0-1): one 2-bank DVE max-reduce off PSUM.
                nc.vector.tensor_reduce(
                    out=y_fr[g][:, :, jb, :],
                    in_=ptile[:, 0:2],
                    axis=mybir.AxisListType.X,
                    op=mybir.AluOpType.max,
                )
                # Indicator pair (banks 2-3): ACT relu(S+1) -> bf16 SBUF,
                # DVE bf16 max 32->16, gpsimd add-tree 16->1.
                i1 = tpool.tile([128, 2, 16, 32], BF16, name="i1", tag="i1")
                for i in range(2):
                    nc.scalar.activation(
                        out=i1[:, i], in_=ptile[:, 2 + i], func=RELU, bias=1.0, scale=1.0
                    )
                if not last:
                    q2 = tpool.tile([128, 2, 16, 16], BF16, name="q2", tag="q2")
                    nc.vector.tensor_max(q2[:], i1[:, :, :, 0:16], i1[:, :, :, 16:32])
                    l2 = tpool.tile([128, 2, 16, 8], BF16, name="l2", tag="l2")
                    nc.gpsimd.tensor_add(l2[:], q2[:, :, :, 0:8], q2[:, :, :, 8:16])
                    l3 = tpool.tile([128, 2, 16, 4], BF16, name="l3", tag="l3")
                    nc.gpsimd.tensor_add(l3[:], l2[:, :, :, 0:4], l2[:, :, :, 4:8])
                    l4 = tpool.tile([128, 2, 16, 2], BF16, name="l4", tag="l4")
                    nc.gpsimd.tensor_add(l4[:], l3[:, :, :, 0:2], l3[:, :, :, 2:4])
                    nc.gpsimd.tensor_add(
                        y_fi[g][:, :, jb, :], l4[:, :, :, 0], l4[:, :, :, 1]
                    )
                else:
                    # Tail: finish the staged pair with one DVE bf16 max
                    # reduce (values {0,1}, still nonzero == True).
                    nc.vector.tensor_reduce(
                        out=y_fi[g][:, :, jb, :],
                        in_=i1[:],
                        axis=mybir.AxisListType.X,
                        op=mybir.AluOpType.max,
                    )
                if jb == NJB - 1:
                    # This bt-group's y is complete: compare + pack + store.
                    # Raw pairs: relu(y+1) is the 0/1 indicator (y int <= 0,
                    # 0 == True).  Indicator pairs: relu(y) == y (uint8
                    # counts; the caller booleanizes).
                    nc.scalar.activation(
                        out=y_us[g][:, 0:2, :], in_=y_fr[g][:], func=RELU, bias=1.0, scale=1.0
                    )
                    if not last:
                        nc.scalar.activation(
                            out=y_us[g][:, 2:4, :], in_=y_fi[g][:], func=RELU, bias=0.0, scale=1.0
                        )
                        nc.sync.dma_start(
                            out=y_d[512 * g : 512 * (g + 1), :].rearrange(
                                "(s p) o -> p s o", p=128
                            ),
                            in_=y_us[g][:],
                        )
                    else:
                        # Two half-stores: raw pair ships while the
                        # indicator pair finishes.
                        nc.sync.dma_start(
                            out=y_d[512 * g : 512 * g + 256, :].rearrange(
                                "(s p) o -> p s o", p=128
                            ),
                            in_=y_us[g][:, 0:2, :],
                        )
                        nc.scalar.activation(
                            out=y_us[g][:, 2:4, :], in_=y_fi[g][:], func=RELU, bias=0.0, scale=1.0
                        )
                        nc.sync.dma_start(
                            out=y_d[512 * g + 256 : 512 * (g + 1), :].rearrange(
                                "(s p) o -> p s o", p=128
                            ),
                            in_=y_us[g][:, 2:4, :],
                        )
    return nc


def _get_nc() -> bass.Bass:
    if "nc" not in _CACHE:
        nc = _build_nc()
        nc.finalize()
        _CACHE["nc"] = nc
    return _CACHE["nc"]


def _build_A(weights: np.ndarray) -> np.ndarray:
    w = weights.reshape(J, AND_T).astype(np.int64)
    v = w.reshape(-1)
    j_idx = np.repeat(np.arange(J), AND_T)
    C = np.zeros((KPAD, J), np.float32)
    pos = (v >= 1) & (v <= F)
    neg = v > F
    np.add.at(C, (v[pos] - 1, j_idx[pos]), 1.0)
    np.add.at(C, (v[neg] - 1 - F, j_idx[neg]), -1.0)
    base = (w == 0).sum(1) + neg.reshape(J, AND_T).sum(1)
    padded = (w == 0).all(1)
    C[F, :] = np.where(padded, base - 64.0, base - 16.0).astype(np.float32)
    A8 = C.astype(FP8_NP)
    assert np.array_equal(A8.astype(np.float32), C), "fp8 must be exact"
    return A8


def kernel(x: np.ndarray, weights: np.ndarray) -> np.ndarray:
    x = np.asarray(x)
    weights = np.asarray(weights)
    A8 = _build_A(weights)
    xT8 = np.zeros((KPAD, B), FP8_NP)
    xT8[:F] = (x.T != 0).astype(FP8_NP)
    xT8[F] = np.float32(1.0).astype(FP8_NP)
    in_maps = []
    for c in range(N_CORES):
        bi, ji = divmod(c, JC)
        in_maps.append(
            {
                "xT": np.ascontiguousarray(xT8[:, bi * BS : (bi + 1) * BS]),
                "A": np.ascontiguousarray(A8[:, ji * JS : (ji + 1) * JS]),
            }
        )
    nc = _get_nc()
    res = run_bass_kernel_spmd(nc, in_maps, list(range(N_CORES)))
    y = np.empty((B, OUT), np.uint8)
    for c in range(N_CORES):
        bi, ji = divmod(c, JC)
        y[bi * BS : (bi + 1) * BS, ji * OJ : (ji + 1) * OJ] = res.results[c]["y"]
    return y.astype(bool)


# revision 13
# speedup vs baseline: 1.0437x; 1.0437x over previous
"""Trainium2 Bass kernel for the BinaryLayer problem.

Math: out[b,o] = OR_r ( mask[o,r] AND AND_t x_in[b, w[o,r,t]] ) with
x_in = [1 | x | 1-x].  AND over 16 literals == (sum of literal values == 16).
sum_t lit = base[j] + sum_f C[f,j]*x[b,f]  where for j=(o,r):
  C[f,j]  = (#slots with w==f+1) - (#slots with w==f+1+F)
  base[j] = (#slots with w==0) + (#slots with w>F)
Fold threshold+mask into a constant row: c1[j] = base[j]-16 (valid term)
or base[j]-64 (padded term, all w==0).  Then with S[b,j] = x_aug[b,:]@A[:,j]
(A = [C; c1], x_aug = [x, 1]):  AND true <=> S==0, and since S<=0 always,
out[b,o] = (max_r S[b,o*32+r] >= 0).  All arithmetic is exact small-int
in fp8e4m3 inputs / f32 PSUM accumulation.

Sharding: 4 batch shards x 2 column shards across the 8 cores (core
c = (c//2)-th batch quarter, (c%2)-th j half); cuts per-core HBM bytes
to x 0.5MB + A 2MB vs 0.5+4 for pure data parallel.

Per core: x^T and A ship as fp8 already (host casts the 0/1 ints and
builds A), zero-padded to K=1024 rows so all 4 contraction passes are
uniform DoubleRow subtile pairs and the device needs no masking/casts.
PE runs 8 rounds of (4 passes x 4 batch tiles) fp8 DR matmuls into
4-bank PSUM tiles (107ns per 512-col matmul warm, ~13.7us total - the
critical path).  Draining 2M PSUM values is the second wall (~21us if
DVE did it all, the old bottleneck), so each round's 4 banks split
across all three other engines inside the PE round time:
  - banks 0-1 ("raw" pair): DVE 2-bank tensor_reduce(max) off PSUM;
    values stay ints <= 0 with 0 == term-true.
  - banks 2-3 ("indicator" pair): ACT computes relu(S+1) (exactly the
    0/1 AND-term indicator) PSUM->SBUF bf16; DVE folds 32->16 with one
    2x-mode bf16 tensor_max; gpsimd - which can neither reduce along
    the free axis nor touch PSUM, but CAN pairwise-add in SBUF - sums
    the 16 indicators with a 4-level add tree (OR == count nonzero).
Final compare on ACT: relu(y+1) for raw pairs, relu(y) for indicator
pairs (counts pass through; any nonzero uint8 is True for the caller).
The last round drains on DVE with compares + two half-DMAs chasing at
batch-tile grain to shorten the post-matmul tail.

DMA chunks are sized ~256KB (>=512B contiguous runs for full DMA rate)
and issued in exact consumption order so the PE never stalls after its
first real matmul; dummy matmuls on zeroed scratch cover the prologue
so the PE clock ramp (3us to full speed) completes as the data lands.
"""

import os

os.environ.setdefault("MYCRO_LOCAL_CACHE", "1")

import numpy as np

import concourse.bass as bass
import concourse.bacc as bacc
import concourse.mybir as mybir
from concourse.tile import TileContext
from concourse.bass_utils import run_bass_kernel_spmd

B, F = 4096, 784
OUT, OR_T, AND_T = 128, 32, 16
N_CORES = 8
JC = 2                       # j-dimension shards
BC = N_CORES // JC           # batch shards
BS = B // BC                 # 1024 batch rows per core
NBT = BS // 128              # 8 batch tiles of 128
J = OUT * OR_T               # 4096 (o,r) columns, j = o*32 + r
JS = J // JC                 # 2048 j columns per core
NJB = JS // 512              # 4 column blocks (one f32 PSUM bank each)
OJ = JS // OR_T              # 64 outputs per core
K = F + 1                    # 785 = 784 features + constant row
KPAD = 1024                  # padded to 8 full 128-row subtiles
KSUB = KPAD // 128           # 8
NP = KSUB // 2               # 4 DoubleRow passes
FP8 = mybir.dt.float8e4
FP8_NP = mybir.dt.np(FP8)
F32 = mybir.dt.float32
BF16 = mybir.dt.bfloat16
RELU = mybir.ActivationFunctionType.Relu
N_WU = 16                    # PE warm-up dummy matmuls

_CACHE: dict = {}


def _build_nc(use_double_row: bool = True) -> bass.Bass:
    nc = bacc.Bacc("TRN2")
    xT_d = nc.declare_dram_parameter("xT", [KPAD, BS], FP8, isOutput=False)
    A_d = nc.declare_dram_parameter("A", [KPAD, JS], FP8, isOutput=False)
    y_d = nc.declare_dram_parameter("y", [BS, OJ], mybir.dt.uint8, isOutput=True)

    with TileContext(nc) as tc:
        with (
            tc.tile_pool(name="const", bufs=1) as cpool,
            tc.tile_pool(name="psum", bufs=2, space="PSUM") as ppool,
            tc.tile_pool(name="tree", bufs=2) as tpool,
        ):
            A_sb = cpool.tile([128, KSUB, JS], FP8)
            x_q = cpool.tile([128, KSUB, BS], FP8)
            wu_sb = cpool.tile([128, 384], FP8)
            # Per-pair result accumulators [128, 2bt, jb, 16]: raw pairs
            # (bts 4g+0,1) f32 maxes; indicator pairs (bts 4g+2,3) bf16
            # counts/indicators (nonzero == True).
            y_fr = [cpool.tile([128, 2, NJB, 16], F32, name=f"y_fr{g}") for g in range(2)]
            y_fi = [cpool.tile([128, 2, NJB, 16], BF16, name=f"y_fi{g}") for g in range(2)]
            y_us = [cpool.tile([128, BC, OJ], mybir.dt.uint8, name=f"y_u{g}") for g in range(2)]
            scr_u = cpool.tile([128, 16], mybir.dt.uint8, name="scr_u")

            # Zero the PE warm-up scratch on DVE (idle at t=0; gpsimd pays
            # a slow startup) so dummy matmuls start by ~1us, and preload
            # the ACT Relu table early so the first real activation doesn't
            # pay the 1.3us table load.
            nc.vector.memset(wu_sb[:], 0.0)
            nc.scalar.activation(
                out=scr_u[:], in_=wu_sb[:, 0:16], func=RELU, bias=1.0, scale=1.0
            )

            # DMA chunks in exact consumption order.  Rounds are
            # (jb, bt-group); passes consume x/A subtile pairs in order, so
            # ship [rows 0:512] then [512:1024] halves (~256KB each); all
            # contiguous runs are 512B so the DMA bus runs at full rate.
            def xdma(h, g):
                rows = slice(512 * h, 512 * (h + 1))
                cols = slice(512 * g, 512 * (g + 1))
                nc.sync.dma_start(
                    out=x_q[:, 4 * h : 4 * h + 4, cols],
                    in_=xT_d[rows, cols].rearrange("(s p) b -> p s b", p=128),
                )

            def adma(jb, h):
                rows = slice(512 * h, 512 * (h + 1))
                cols = slice(512 * jb, 512 * (jb + 1))
                nc.sync.dma_start(
                    out=A_sb[:, 4 * h : 4 * h + 4, cols],
                    in_=A_d[rows, cols].rearrange("(s p) j -> p s j", p=128),
                )

            xdma(0, 0)
            adma(0, 0)
            xdma(1, 0)
            adma(0, 1)
            xdma(0, 1)
            xdma(1, 1)
            for jb in range(1, NJB):
                adma(jb, 0)
                adma(jb, 1)

            # Dummy matmuls on zeroed scratch: keep the PE busy (the clock
            # ramp needs ~3us of sustained work to reach 2.4GHz) while the
            # first x/A chunks are in flight.  wtile is created before the
            # first round tile so the 2-buffer PSUM rotation pairs rounds
            # (r, r+2), not (r, r+1).
            wtile = ppool.tile([128, 4, 16, 32], F32, name="wu_ps", tag="ps")
            for _ in range(N_WU):
                nc.tensor.matmul(
                    wtile[:, 0, 0:8, :], wu_sb[:, 0:128], wu_sb[:, 128:384],
                    start=True, stop=True,
                )

            rounds = [(jb, g) for jb in range(NJB) for g in range(2)]
            for jb, g in rounds:
                last = (jb, g) == rounds[-1]
                ptile = ppool.tile([128, 4, 16, 32], F32, name="ps", tag="ps")
                jsl = slice(512 * jb, 512 * (jb + 1))
                # Pass-major order matches the DMA chunk arrival order; the
                # last round is bt-major so its banks finish staggered and
                # the drain/compare/store chain starts early.
                mms = [(p, i) for p in range(NP) for i in range(4)]
                if last:
                    mms = [(p, i) for i in range(4) for p in range(NP)]
                for p, i in mms:
                    ssl = slice(2 * p, 2 * p + 2)
                    bt = 4 * g + i
                    bsl = slice(128 * bt, 128 * (bt + 1))
                    nc.tensor.matmul(
                        ptile[:, i],
                        x_q[:, ssl, bsl],
                        A_sb[:, ssl, jsl],
                        start=(p == 0),
                        stop=(p == NP - 1),
                        perf_mode=mybir.MatmulPerfMode.DoubleRow,
                    )
                # Raw pair (banks 0-1): one 2-bank DVE max-reduce off PSUM.
                nc.vector.tensor_reduce(
                    out=y_fr[g][:, :, jb, :],
                    in_=ptile[:, 0:2],
                    axis=mybir.AxisListType.X,
                    op=mybir.AluOpType.max,
                )
                # Indicator pair (banks 2-3): ACT relu(S+1) -> bf16 SBUF,
                # DVE bf16 max 32->16, gpsimd add-tree 16->1.  One ACT op
                # for both banks in normal rounds (per-op overhead ~220ns);
                # per-bank in the last round so the chain starts before the
                # final bank lands.
                i1 = tpool.tile([128, 2, 16, 32], BF16, name="i1", tag="i1")
                if not last:
                    nc.scalar.activation(
                        out=i1[:], in_=ptile[:, 2:4], func=RELU, bias=1.0, scale=1.0
                    )
                else:
                    for i in range(2):
                        nc.scalar.activation(
                            out=i1[:, i], in_=ptile[:, 2 + i], func=RELU, bias=1.0, scale=1.0
                        )
                if not last:
                    q2 = tpool.tile([128, 2, 16, 16], BF16, name="q2", tag="q2")
                    nc.vector.tensor_max(q2[:], i1[:, :, :, 0:16], i1[:, :, :, 16:32])
                    l2 = tpool.tile([128, 2, 16, 8], BF16, name="l2", tag="l2")
                    nc.gpsimd.tensor_add(l2[:], q2[:, :, :, 0:8], q2[:, :, :, 8:16])
                    l3 = tpool.tile([128, 2, 16, 4], BF16, name="l3", tag="l3")
                    nc.gpsimd.tensor_add(l3[:], l2[:, :, :, 0:4], l2[:, :, :, 4:8])
                    l4 = tpool.tile([128, 2, 16, 2], BF16, name="l4", tag="l4")
                    nc.gpsimd.tensor_add(l4[:], l3[:, :, :, 0:2], l3[:, :, :, 2:4])
                    nc.gpsimd.tensor_add(
                        y_fi[g][:, :, jb, :], l4[:, :, :, 0], l4[:, :, :, 1]
                    )
                else:
                    # Tail: finish the staged pair with one DVE bf16 max
                    # reduce (values {0,1}, still nonzero == True).
                    nc.vector.tensor_reduce(
                        out=y_fi[g][:, :, jb, :],
                        in_=i1[:],
                        axis=mybir.AxisListType.X,
                        op=mybir.AluOpType.max,
                    )
                if jb == NJB - 1:
                    # This bt-group's y is complete: compare + pack + store.
                    # Raw pairs: relu(y+1) is the 0/1 indicator (y int <= 0,
                    # 0 == True).  Indicator pairs: relu(y) == y (uint8
                    # counts; the caller booleanizes).
                    nc.scalar.activation(
                        out=y_us[g][:, 0:2, :], in_=y_fr[g][:], func=RELU, bias=1.0, scale=1.0
                    )
                    if not last:
                        nc.scalar.activation(
                            out=y_us[g][:, 2:4, :], in_=y_fi[g][:], func=RELU, bias=0.0, scale=1.0
                        )
                        nc.sync.dma_start(
                            out=y_d[512 * g : 512 * (g + 1), :].rearrange(
                                "(s p) o -> p s o", p=128
                            ),
                            in_=y_us[g][:],
                        )
                    else:
                        # Two half-stores: raw pair ships while the
                        # indicator pair finishes.
                        nc.sync.dma_start(
                            out=y_d[512 * g : 512 * g + 256, :].rearrange(
                                "(s p) o -> p s o", p=128
                            ),
                            in_=y_us[g][:, 0:2, :],
                        )
                        nc.scalar.activation(
                            out=y_us[g][:, 2:4, :], in_=y_fi[g][:], func=RELU, bias=0.0, scale=1.0
                        )
                        nc.sync.dma_start(
                            out=y_d[512 * g + 256 : 512 * (g + 1), :].rearrange(
                                "(s p) o -> p s o", p=128
                            ),
                            in_=y_us[g][:, 2:4, :],
                        )
    return nc


def _get_nc() -> bass.Bass:
    if "nc" not in _CACHE:
        nc = _build_nc()
        nc.finalize()
        _CACHE["nc"] = nc
    return _CACHE["nc"]


def _build_A(weights: np.ndarray) -> np.ndarray:
    w = weights.reshape(J, AND_T).astype(np.int64)
    v = w.reshape(-1)
    j_idx = np.repeat(np.arange(J), AND_T)
    C = np.zeros((KPAD, J), np.float32)
    pos = (v >= 1) & (v <= F)
    neg = v > F
    np.add.at(C, (v[pos] - 1, j_idx[pos]), 1.0)
    np.add.at(C, (v[neg] - 1 - F, j_idx[neg]), -1.0)
    base = (w == 0).sum(1) + neg.reshape(J, AND_T).sum(1)
    padded = (w == 0).all(1)
    C[F, :] = np.where(padded, base - 64.0, base - 16.0).astype(np.float32)
    A8 = C.astype(FP8_NP)
    assert np.array_equal(A8.astype(np.float32), C), "fp8 must be exact"
    return A8


def kernel(x: np.ndarray, weights: np.ndarray) -> np.ndarray:
    x = np.asarray(x)
    weights = np.asarray(weights)
    A8 = _build_A(weights)
    xT8 = np.zeros((KPAD, B), FP8_NP)
    xT8[:F] = (x.T != 0).astype(FP8_NP)
    xT8[F] = np.float32(1.0).astype(FP8_NP)
    in_maps = []
    for c in range(N_CORES):
        bi, ji = divmod(c, JC)
        in_maps.append(
            {
                "xT": np.ascontiguousarray(xT8[:, bi * BS : (bi + 1) * BS]),
                "A": np.ascontiguousarray(A8[:, ji * JS : (ji + 1) * JS]),
            }
        )
    nc = _get_nc()
    res = run_bass_kernel_spmd(nc, in_maps, list(range(N_CORES)))
    y = np.empty((B, OUT), np.uint8)
    for c in range(N_CORES):
        bi, ji = divmod(c, JC)
        y[bi * BS : (bi + 1) * BS, ji * OJ : (ji + 1) * OJ] = res.results[c]["y"]
    return y.astype(bool)
